# revision 1
# baseline (speedup 1.0000x reference)
"""Trainium2 Bass kernel for nn_BandSplit (banded matmul, fp8 x, variable band).

The reference pipeline (gather -> mask -> per-band linear -> linear -> mask ->
scatter_add -> OLA) is linear in x and collapses to ONE banded matrix multiply
in the interleaved linear space lin = f*4 + c:

    out_lin[l', r] = sum_l A[l, l'] * x_lin[l, r]        (r = b*T + t rows)

A is built on the host from the (small) weight inputs.  The band support of
each 128-wide output tile varies from 128 rows (low mel bands) to ~320 (high):
instead of a fixed 3-diagonal blocking, each out-tile j contracts over
nd(j) = ceil(support_width/128) slices of 128 input rows placed at arbitrary
(host-chosen) offsets, with overlap rows zeroed in the weights.  nd is 1-2 for
24 tiles and 3 for 8 tiles, so each core gets a uniform SPMD slot pattern
(2,2,2,3) = 9 weight blocks: [pair of adjacent tiles | single tile | one nd-3
tile], sharing x slices within the pair.  Per-core DMA: 8 x-slices.

Dtypes: x is quantized host-side to fp8 E3M4 (scale SX folded into A; ~1.3%
rel err on N(0,1) data), weights fp16, PSUM fp32.  3/4 of the output columns
are stored fp8 E3M4 (x SO, divided out on the host), the rest fp16; measured
rel err 1.76e-2 vs the 2e-2 gate.  Bias image and the 4 outputs above lin
4096 (f-bin 1024) are per-(c,f) host-side constants / tiny residuals.

Per-core steady-state budget: PE 9 block-streams x 2048 cols = 18.4K cycles
~ 8.0us at full clock; DMA ~3.57 MB at ~300 GB/s (bound).  Three parallel
DMA queues -- loads on SP, fp8 stores on ACT, fp16 stores on gpsimd/SWDGE --
with all PSUM->SBUF copies on the DVE, so consecutive bodies pipeline with
no in-order-queue coupling (loads must never queue behind stores and vice
versa; every other queue permutation measured slower).  Measured 12.36
us/body (unroll=8 replay), rel err 1.763e-2.
"""

import numpy as np
import ml_dtypes

# ---- problem constants (hardcoded; harness supplies matching inputs) ----
B, C, T, F = 4, 4, 512, 1025
KB, WMAX = 256, 33
L = F * C                 # 4100 linear positions
R = B * T                 # 2048 rows (b, t)
NT_DEV = 32               # device out tiles (lin 0..4096); rest host residual
RES_LO = NT_DEV * 128     # 4096
NCORES = 8
CHUNK = 512               # PSUM bank (fp32) free-dim limit
NCHUNK = R // CHUNK       # 4

# uniform per-core slot structure: [pair lo, pair hi, single, high]
NDP = (2, 2, 2, 3)                    # weight blocks per slot
SMAP = ((0, 1), (1, 2), (3, 4), (5, 6, 7))   # x-slice index per block
NSL = 8                               # x slices per core (slice 7 is 64 rows)
NSL7 = 7                              # 128-row slices in the interleaved blob
NBLK = sum(NDP)                       # 9 weight blocks per core
NTPC = len(NDP)                       # 4 out tiles per core

# out-tile assignment per core: (pair0, pair0+1, single, high)
PAIRS = [0, 2, 4, 6, 8, 10, 12, 30]
SINGLES = [14, 15, 16, 17, 18, 19, 20, 24]
HIGHS = [21, 22, 23, 25, 26, 27, 28, 29]
CORE_TILES = [(PAIRS[c], PAIRS[c] + 1, SINGLES[c], HIGHS[c])
              for c in range(NCORES)]

SX_TARGET = 14.8          # fp8 e3m4 max normal is 15.5; leave clip margin
SO = 3.0                  # fp8 out scale (out absmax ~2.3, 15.5/3=5.2 cap)
# column chunks: first CHW16 cols stored fp16 (stores overlap compute), the
# rest fp8; the final rounds are narrow so the drain tail is short
CHW = (512, 512, 512, 512)
CHOFF = tuple(int(sum(CHW[:i])) for i in range(len(CHW) + 1))
N16 = 1                   # chunks 0..N16-1 are fp16
R16 = CHOFF[N16]          # 512
R8 = R - R16

F8 = ml_dtypes.float8_e3m4

_prog_cache = {}


def _build_program(loop_iters=1, unroll=4):
    """loop_iters counts BODY executions; the hardware loop runs
    loop_iters/unroll iterations of `unroll` pipelined bodies (the revolving
    bufs=2 pools overlap consecutive bodies; the all-engine barrier sits on
    the loop back-edge only)."""
    import concourse.bacc as bacc
    import concourse.tile as tile
    import concourse.mybir as mybir

    if loop_iters % unroll:
        unroll = 1
    key = (loop_iters, unroll)
    if key in _prog_cache:
        return _prog_cache[key]

    f32 = mybir.dt.float32
    f16 = mybir.dt.float16
    f8 = mybir.dt.float8e3

    nc = bacc.Bacc("TRN2", target_bir_lowering=False, debug=False,
                   num_devices=NCORES)
    xin = nc.dram_tensor("xin", [128, NSL * R], f8, kind="ExternalInput").ap()
    wts = nc.dram_tensor("wts", [128, NBLK * 128], f16,
                         kind="ExternalInput").ap()
    out8 = nc.dram_tensor("out8", [NTPC * 128, R8], f8,
                          kind="ExternalOutput").ap()
    out16 = nc.dram_tensor("out16", [NTPC * 128, R16], f16,
                           kind="ExternalOutput").ap()

    blk0 = [sum(NDP[:t]) for t in range(NTPC)]   # first block of each slot

    # which (slice, chunk) x tiles each slot's chunk-ch matmuls consume;
    # load order: for each chunk, w-slot pieces interleaved with the slices
    # that slot needs, so matmul (slot0, ch0) only waits for ~192 KB.
    with tile.TileContext(nc) as tc:
        with (
            tc.tile_pool(name="xp", bufs=2) as xp,
            tc.tile_pool(name="wp", bufs=2) as wp,
            tc.tile_pool(name="y8p", bufs=2) as y8p,
            tc.tile_pool(name="y16p", bufs=2) as y16p,
            tc.tile_pool(name="pp", bufs=8, space="PSUM") as pp,
        ):
            # x DRAM layout is (chunk, slice)-interleaved: col block
            # (ch*NSL + i)*CHUNK holds chunk ch of slice i, so each chunk is
            # one contiguous ~0.5 MB load descriptor (HWDGE queue cost is per
            # descriptor) and compute can start after w0 + chunk 0 (~0.7 MB).
            # Matmuls run chunk-major so each chunk's compute chases its load.
            def body(_iv=None):
                xt = xp.tile([128, NSL7 * R], f8, tag="x")
                xs7 = xp.tile([64, R], f8, tag="x7")
                wt0 = wp.tile([128, NDP[0] * 128], f16, tag="w0")
                wtr = wp.tile([128, (NBLK - NDP[0]) * 128], f16, tag="wr")
                # steady-state bodies pipeline; one ~0.5 MB descriptor per
                # column chunk keeps the load sem granularity matched to the
                # compute rounds (coarser couples the pipeline, finer wastes
                # ~625 ns of DGE queue time per descriptor)
                xo = [NSL7 * o for o in CHOFF]
                nc.sync.dma_start(wt0[:], wts[:, :NDP[0] * 128])
                nc.sync.dma_start(xt[:, 0:xo[1]], xin[:, 0:xo[1]])
                nc.sync.dma_start(wtr[:], wts[:, NDP[0] * 128:])
                # slice 7 (64 rows, used by the high slot's 3rd block) lives
                # in xin cols [7R, 8R) partitions [0:64); one load
                nc.sync.dma_start(xs7[:], xin[0:64, NSL7 * R:NSL7 * R + R])
                for ch in range(1, len(CHW)):
                    nc.sync.dma_start(xt[:, xo[ch]:xo[ch + 1]],
                                      xin[:, xo[ch]:xo[ch + 1]])

                def wblk(t, b):
                    if t == 0:
                        return wt0[:, b * 128:(b + 1) * 128]
                    blk = (blk0[t] - NDP[0] + b) * 128
                    return wtr[:, blk:blk + 128]

                # per-tile y buffers.  Tiles 0-1 are owned by the DVE, tiles
                # 2-3 by the ACT engine: the owner does the tile's PSUM
                # copies AND triggers its store DMA from its own queue, so
                # stores follow copies by program order on a queue separate
                # from the loads (SP queue) -- consecutive loop bodies can
                # overlap.
                y8s = [y8p.tile([128, R8], f8, tag=f"y8_{t}",
                                name=f"y8_{t}") for t in range(NTPC)]
                y16s = [y16p.tile([128, R16], f16, tag=f"y16_{t}",
                                  name=f"y16_{t}") for t in range(NTPC)]

                def owner(t):
                    # DVE copies tiles 0-2 (it is ~1.6x faster than ACT and
                    # ACT also triggers all the store DMAs); only SP/ACT can
                    # start HWDGE DMAs, so stores go on the ACT queue
                    # (separate from the SP load queue)
                    return nc.vector if t < 4 else nc.scalar

                def psum_copy(t, dst, ps, scale):
                    eng = owner(t)
                    if scale is None:
                        if eng is nc.scalar:
                            nc.scalar.copy(dst, ps)
                        else:
                            eng.tensor_copy(dst, ps)
                    elif eng is nc.scalar:
                        nc.scalar.mul(dst, ps, scale)
                    else:
                        eng.tensor_scalar_mul(dst, ps, scale)

                nch = len(CHW)
                for ch in range(nch):
                    w = CHW[ch]
                    order = range(NTPC) if ch == 0 else (3, 0, 1, 2)
                    for t in order:
                        ps = pp.tile([128, w], f32, tag="ps")
                        nd = NDP[t]
                        for b in range(nd):
                            if t == 3 and b == 2:
                                lhs = wblk(t, b)[0:64, :]
                                rhs = xs7[:, CHOFF[ch]:CHOFF[ch] + w]
                            else:
                                lhs = wblk(t, b)
                                c0 = (xo[ch] + SMAP[t][b] * w)
                                rhs = xt[:, c0:c0 + w]
                            nc.tensor.matmul(
                                ps[:], lhs, rhs,
                                start=(b == 0), stop=(b == nd - 1),
                            )
                        if ch < N16:
                            psum_copy(t, y16s[t][:, CHOFF[ch]:CHOFF[ch] + w],
                                      ps[:], None)
                            if ch == N16 - 1:
                                # fp16 stores ride the gpsimd SWDGE queue --
                                # a third DMA path alongside SP (loads) and
                                # ACT (fp8 stores), measured ~0.6 us/body
                                nc.gpsimd.dma_start(
                                    out16[t * 128:(t + 1) * 128, :],
                                    y16s[t][:])
                        else:
                            c8 = CHOFF[ch] - R16
                            psum_copy(t, y8s[t][:, c8:c8 + w], ps[:], SO)
                            if ch == nch - 1:
                                nc.scalar.dma_start(
                                    out8[t * 128:(t + 1) * 128, :], y8s[t][:])

            if loop_iters == 1:
                body()
            else:
                with tc.For_i(0, loop_iters // unroll, 1) as _i:
                    for _u in range(unroll):
                        body(_i)

    nc.compile()
    _prog_cache[key] = nc
    return nc


def _build_A(pre_weight, pre_bias, post_weight, post_bias, mask, ola_window,
             f_idxes):
    """Host: banded operator A[in_lin, out_lin] and the bias image (C, F)."""
    fi = f_idxes.reshape(KB, WMAX).astype(np.int64)
    mk = mask.reshape(KB, WMAX).astype(np.float32)
    ola = ola_window.astype(np.float32)

    mrow = np.repeat(mk, C, axis=1)                     # (KB, WMAX*C)
    inv_ola = np.where(ola != 0, 1.0 / ola, 0.0)
    ola_cols = inv_ola[fi]                              # (KB, WMAX)
    mcol = np.repeat(mk * ola_cols, C, axis=1)          # (KB, WMAX*C)

    w1 = pre_weight * mrow[:, :, None]                  # (KB, D, 128)
    w2 = post_weight * mcol[:, None, :]                 # (KB, 128, D)
    Mk = np.matmul(w1, w2)                              # (KB, D, D) fp32

    LPAD = ((L + 127) // 128) * 128
    A = np.zeros((LPAD, LPAD), np.float32)
    lin = (fi[:, :, None] * C + np.arange(C)[None, None, :]).reshape(KB, -1)
    for k in range(KB):
        idx = lin[k]
        A[np.ix_(idx, idx)] += Mk[k]

    by = (np.einsum('ko,koj->kj', pre_bias, post_weight) + post_bias)
    by = by * mcol
    bias_img = np.zeros((C, F), np.float32)
    np.add.at(bias_img,
              (np.tile(np.arange(C), (KB, WMAX, 1)).reshape(KB, -1),
               np.repeat(fi, C, axis=1)),
              by)
    return A, bias_img


def _plan_slices(A):
    """Per-core x-slice offsets + per-block (offset, new-row mask) coverage.

    Returns (slice_offs, blocks): slice_offs[core][NSL]; blocks[core] is a
    list of NBLK (tile_j, off, newmask[128]) entries (newmask selects rows of
    the slice not already covered by earlier blocks of the same tile).
    """
    sup = []
    nzc = A[:L, :RES_LO] != 0
    for j in range(NT_DEV):
        rows = np.nonzero(nzc[:, 128 * j:128 * (j + 1)].any(axis=1))[0]
        sup.append((int(rows.min()), int(rows.max())))

    def clamp(o):
        return max(0, min(L - 128, o))

    slice_offs, blocks = [], []
    for c in range(NCORES):
        p0, p1, s, h = CORE_TILES[c]
        offs = [0] * NSL
        lo0, hi0 = sup[p0]
        lo1, hi1 = sup[p1]
        assert hi0 - lo0 < 256 and hi1 - lo1 < 256
        offs[0] = clamp(lo0)
        # slice 1 serves the tail of p0 AND the head of p1: any offset in
        # [max(hi0-127, hi1-255), min(offs0+128, lo1)] works (pair span<384)
        s1_lo, s1_hi = max(hi0 - 127, hi1 - 255), min(offs[0] + 128, lo1)
        assert s1_lo <= s1_hi, (c, p0, p1, s1_lo, s1_hi)
        offs[1] = clamp(s1_hi)
        assert offs[1] <= offs[0] + 128 and offs[1] + 128 > hi0
        offs[2] = clamp(max(hi1 - 127, offs[1]))
        assert offs[2] <= offs[1] + 128 and offs[2] + 128 > hi1
        lo2, hi2 = sup[s]
        assert hi2 - lo2 < 256
        offs[3] = clamp(lo2)
        offs[4] = clamp(max(hi2 - 127, offs[3]))
        assert offs[4] <= offs[3] + 128 and offs[4] + 128 > hi2
        # high slot: fixed window [lo, lo+320) -- slices at lo, lo+128 and
        # a 64-row slice at lo+256 (every high support is <= 320 wide)
        lo3, hi3 = sup[h]
        assert hi3 - lo3 < 320, (h, lo3, hi3)
        offs[5] = clamp(min(lo3, L - 320))
        offs[6] = offs[5] + 128
        offs[7] = offs[5] + 256
        assert offs[7] + 64 > hi3

        blks = []
        for t, j in enumerate((p0, p1, s, h)):
            covered = np.zeros(L, bool)
            for b in range(NDP[t]):
                o = offs[SMAP[t][b]]
                rows = 64 if (t == 3 and b == 2) else 128
                new = np.zeros(128, bool)
                new[:rows] = ~covered[o:o + rows]
                blks.append((j, o, new.copy()))
                covered[o:o + rows] = True
        slice_offs.append(offs)
        blocks.append(blks)
    return slice_offs, blocks


def _shard_inputs(x, A):
    """Per-core in_maps plus host-side residual rows (lin 4096..4099)."""
    X = np.ascontiguousarray(
        np.asarray(x, np.float32).transpose(3, 1, 0, 2).reshape(L, R))
    sx = SX_TARGET / max(float(np.abs(X).max()), 1e-30)
    Xq = np.clip(X * sx, -15.5, 15.5).astype(F8)

    slice_offs, blocks = _plan_slices(A)
    in_maps = []
    for c in range(NCORES):
        # (chunk, slice)-interleaved column layout for slices 0-6, matching
        # _build_program; the 64-row slice 7 sits at cols [7R, 8R) rows 0-63
        xin = np.zeros((128, NSL * R), F8)
        for i, o in enumerate(slice_offs[c][:NSL7]):
            for ch, w in enumerate(CHW):
                d0 = NSL7 * CHOFF[ch] + i * w
                xin[:, d0:d0 + w] = Xq[o:o + 128, CHOFF[ch]:CHOFF[ch] + w]
        o7 = slice_offs[c][7]
        xin[0:64, NSL7 * R:NSL7 * R + R] = Xq[o7:o7 + 64]
        wts = np.zeros((128, NBLK * 128), np.float32)
        for bi, (j, o, new) in enumerate(blocks[c]):
            wblk = A[o:o + 128, j * 128:(j + 1) * 128] * new[:, None]
            wts[:, bi * 128:(bi + 1) * 128] = wblk
        wts = (wts / sx).astype(np.float16)
        in_maps.append({"xin": xin, "wts": wts})

    # host residual: out lins [4096, 4100) (f-bin 1024), exact in fp32
    nzc = A[:L, RES_LO:L] != 0
    ri = int(np.nonzero(nzc.any(axis=1))[0].min())
    residual = A[ri:L, RES_LO:L].T @ X[ri:L]             # [4, R] fp32
    return in_maps, residual


def _gather_output(results, bias_img, residual):
    out_lin = np.zeros((L, R), np.float32)
    for c in range(NCORES):
        o8 = np.asarray(results[c]["out8"]).astype(np.float32) / SO
        o16 = np.asarray(results[c]["out16"]).astype(np.float32)
        for t, j in enumerate(CORE_TILES[c]):
            out_lin[j * 128:(j + 1) * 128, :R16] = o16[t * 128:(t + 1) * 128]
            out_lin[j * 128:(j + 1) * 128, R16:] = o8[t * 128:(t + 1) * 128]
    out_lin[RES_LO:L] = residual
    out = out_lin.reshape(F, C, B, T).transpose(2, 1, 3, 0)
    out = np.ascontiguousarray(out) + bias_img[None, :, None, :]
    return out.astype(np.float32)


def _run_on_device(in_maps, loop_iters=1):
    from concourse.bass_utils import run_bass_kernel_spmd
    nc = _build_program(loop_iters)
    res = run_bass_kernel_spmd(nc, in_maps, list(range(NCORES)))
    return res.results


def kernel(x, pre_weight, pre_bias, post_weight, post_bias, mask, ola_window,
           f_idxes):
    x = np.asarray(x, np.float32)
    pre_weight = np.asarray(pre_weight, np.float32)
    pre_bias = np.asarray(pre_bias, np.float32)
    post_weight = np.asarray(post_weight, np.float32)
    post_bias = np.asarray(post_bias, np.float32)
    mask = np.asarray(mask, np.float32)
    ola_window = np.asarray(ola_window, np.float32)
    f_idxes = np.asarray(f_idxes)

    A, bias_img = _build_A(pre_weight, pre_bias, post_weight, post_bias,
                           mask, ola_window, f_idxes)
    in_maps, residual = _shard_inputs(x, A)
    results = _run_on_device(in_maps)
    return _gather_output(results, bias_img, residual)



# revision 4
# speedup vs baseline: 3.2265x; 3.2265x over previous
"""Trainium2 Bass kernel for nn_BandSplit (banded matmul, fp8 x, chain slices).

The reference pipeline (gather -> mask -> per-band linear -> linear -> mask ->
scatter_add -> OLA) is linear in x and collapses to ONE banded matrix multiply
in the interleaved linear space lin = f*4 + c:

    out_lin[l', r] = sum_l A[l, l'] * x_lin[l, r]        (r = b*T + t rows)

A is built on the host from the (small) weight inputs.  Each core owns 4
ADJACENT 128-wide output tiles (core c -> tiles 4c..4c+3); their band supports
overlap strongly, so one shared "dense chain" of NSL=6 x-slices (128 rows
each, host-chosen offsets, consecutive gaps <= 128) covers all four supports:
tile t contracts over chain slices (t, t+1, t+2) with duplicate/out-of-band
rows zeroed in the weights.  That is 6 x-slice loads per core (vs 8 in the
per-band-slot layout) at the cost of a uniform NDP=(3,3,3,3)=12 weight blocks.

Dtypes: x quantized host-side to fp8 E3M4 (scale SX folded into A), weights
fp16, PSUM fp32, ALL outputs stored fp8 E3M4 (x SO, divided out on the host).
Bias image and the 4 output lins above 4096 (f-bin 1024) are host-side.

Per-core steady-state budget: PE 12 block-streams x 2048 cols = 24.6K cycles
~ 10.2us at 2.4GHz; DMA 3.01 MB (x 1.57 + w 0.39 + out 1.05).  Loads ride the
SP HWDGE queue, fp8 stores split across the ACT HWDGE and gpsimd SWDGE queues,
PSUM->SBUF copies on the DVE, so consecutive bodies pipeline with no
in-order-queue coupling.
"""

import numpy as np
import ml_dtypes

# ---- problem constants (hardcoded; harness supplies matching inputs) ----
B, C, T, F = 4, 4, 512, 1025
KB, WMAX = 256, 33
L = F * C                 # 4100 linear positions
R = B * T                 # 2048 rows (b, t)
NT_DEV = 32               # device out tiles (lin 0..4096); rest host residual
RES_LO = NT_DEV * 128     # 4096
NCORES = 8
CHUNK = 512               # PSUM bank (fp32) free-dim limit
NCH = R // CHUNK          # 4

# dense-chain slot structure: NSL slices, tile t reads slices (t, t+1, t+2)
NSL = 6                               # x slices per core (128 rows each)
NTPC = 4                              # out tiles per core
NDP = (3, 3, 3, 3)                    # weight blocks per tile
SMAP = tuple(tuple(range(t, t + 3)) for t in range(NTPC))
NBLK = sum(NDP)                       # 12 weight blocks per core

CORE_TILES = [tuple(range(4 * c, 4 * c + 4)) for c in range(NCORES)]

SX_TARGET = 14.8          # fp8 e3m4 max normal is 15.5; leave clip margin
SO = 5.0                  # fp8 out scale (out absmax ~2.41 -> 12.1 < 15.5)

F8 = ml_dtypes.float8_e3m4

_prog_cache = {}


def _build_program(loop_iters=1, unroll=4):
    """loop_iters counts BODY executions; the hardware loop runs
    loop_iters/unroll iterations of `unroll` pipelined bodies (the revolving
    bufs=2 pools overlap consecutive bodies; the all-engine barrier sits on
    the loop back-edge only)."""
    import concourse.bacc as bacc
    import concourse.tile as tile
    import concourse.mybir as mybir

    if unroll and loop_iters % unroll:
        unroll = 1
    key = (loop_iters, unroll)
    if key in _prog_cache:
        return _prog_cache[key]

    f32 = mybir.dt.float32
    f16 = mybir.dt.float16
    f8 = mybir.dt.float8e3

    nc = bacc.Bacc("TRN2", target_bir_lowering=False, debug=False,
                   num_devices=NCORES)
    xin = nc.dram_tensor("xin", [128, NSL * R], f8, kind="ExternalInput").ap()
    wts = nc.dram_tensor("wts", [128, NBLK * 128], f16,
                         kind="ExternalInput").ap()
    out8 = nc.dram_tensor("out8", [NTPC * 128, R], f8,
                          kind="ExternalOutput").ap()

    with tile.TileContext(nc) as tc:
        with (
            tc.tile_pool(name="xp", bufs=2) as xp,
            tc.tile_pool(name="wp", bufs=2) as wp,
            tc.tile_pool(name="y8p", bufs=2) as y8p,
            tc.tile_pool(name="pp", bufs=8, space="PSUM") as pp,
        ):
            # x DRAM layout is (chunk, slice)-interleaved: col block
            # (ch*NSL + s)*CHUNK holds chunk ch of slice s, so each chunk is
            # one contiguous 384 KB load descriptor and compute can start
            # after w0 + chunk 0.  Matmuls run chunk-major so each chunk's
            # compute chases its load.
            def body(_iv=None):
                xt = xp.tile([128, NSL * R], f8, tag="x")
                wt0 = wp.tile([128, NDP[0] * 128], f16, tag="w0")
                wtr = wp.tile([128, (NBLK - NDP[0]) * 128], f16, tag="wr")
                xo = [NSL * CHUNK * ch for ch in range(NCH + 1)]
                nc.sync.dma_start(wt0[:], wts[:, :NDP[0] * 128])
                nc.sync.dma_start(xt[:, 0:xo[1]], xin[:, 0:xo[1]])
                nc.sync.dma_start(wtr[:], wts[:, NDP[0] * 128:])
                for ch in range(1, NCH):
                    nc.sync.dma_start(xt[:, xo[ch]:xo[ch + 1]],
                                      xin[:, xo[ch]:xo[ch + 1]])

                def wblk(t, b):
                    if t == 0:
                        return wt0[:, b * 128:(b + 1) * 128]
                    blk = (sum(NDP[:t]) - NDP[0] + b) * 128
                    return wtr[:, blk:blk + 128]

                y8s = [y8p.tile([128, R], f8, tag=f"y8_{t}",
                                name=f"y8_{t}") for t in range(NTPC)]

                for ch in range(NCH):
                    order = range(NTPC) if ch == 0 else (3, 0, 1, 2)
                    for t in order:
                        ps = pp.tile([128, CHUNK], f32, tag="ps")
                        for b in range(NDP[t]):
                            c0 = xo[ch] + SMAP[t][b] * CHUNK
                            nc.tensor.matmul(
                                ps[:], wblk(t, b), xt[:, c0:c0 + CHUNK],
                                start=(b == 0), stop=(b == NDP[t] - 1),
                            )
                        # DVE does all PSUM->SBUF copies (with the SO scale
                        # folded in); stores ride the ACT HWDGE queue (tiles
                        # 0-1) and the gpsimd SWDGE queue (tiles 2-3), both
                        # separate from the SP load queue so consecutive
                        # bodies overlap.
                        nc.vector.tensor_scalar_mul(
                            y8s[t][:, ch * CHUNK:(ch + 1) * CHUNK],
                            ps[:], SO)
                        if ch == NCH - 1:
                            eng = nc.scalar if t < 2 else nc.gpsimd
                            eng.dma_start(
                                out8[t * 128:(t + 1) * 128, :], y8s[t][:])

            if loop_iters == 1:
                body()
            elif unroll == 0:
                # straight-line replay (no For_i): TimelineSim cannot follow
                # reg-mode branches, so simulation uses this variant
                for _u in range(loop_iters):
                    body()
            else:
                with tc.For_i(0, loop_iters // unroll, 1) as _i:
                    for _u in range(unroll):
                        body(_i)

    nc.compile()
    _prog_cache[key] = nc
    return nc


def _build_A(pre_weight, pre_bias, post_weight, post_bias, mask, ola_window,
             f_idxes):
    """Host: banded operator A[in_lin, out_lin] and the bias image (C, F)."""
    fi = f_idxes.reshape(KB, WMAX).astype(np.int64)
    mk = mask.reshape(KB, WMAX).astype(np.float32)
    ola = ola_window.astype(np.float32)

    mrow = np.repeat(mk, C, axis=1)                     # (KB, WMAX*C)
    inv_ola = np.where(ola != 0, 1.0 / ola, 0.0)
    ola_cols = inv_ola[fi]                              # (KB, WMAX)
    mcol = np.repeat(mk * ola_cols, C, axis=1)          # (KB, WMAX*C)

    w1 = pre_weight * mrow[:, :, None]                  # (KB, D, 128)
    w2 = post_weight * mcol[:, None, :]                 # (KB, 128, D)
    Mk = np.matmul(w1, w2)                              # (KB, D, D) fp32

    LPAD = ((L + 127) // 128) * 128
    A = np.zeros((LPAD, LPAD), np.float32)
    lin = (fi[:, :, None] * C + np.arange(C)[None, None, :]).reshape(KB, -1)
    for k in range(KB):
        idx = lin[k]
        A[np.ix_(idx, idx)] += Mk[k]

    by = (np.einsum('ko,koj->kj', pre_bias, post_weight) + post_bias)
    by = by * mcol
    bias_img = np.zeros((C, F), np.float32)
    np.add.at(bias_img,
              (np.tile(np.arange(C), (KB, WMAX, 1)).reshape(KB, -1),
               np.repeat(fi, C, axis=1)),
              by)
    return A, bias_img


def _plan_slices(A):
    """Per-core chain slice offsets + per-block (tile, offset, new-row mask).

    Core c owns tiles 4c..4c+3; choose NSL non-decreasing offsets with
    consecutive gaps <= 128 such that tile i's support is inside
    [offs[i], offs[i+2]+128).  Greedy-latest: offs[i] = min(lo_i, prev+128).
    """
    sup = []
    nzc = A[:L, :RES_LO] != 0
    for j in range(NT_DEV):
        rows = np.nonzero(nzc[:, 128 * j:128 * (j + 1)].any(axis=1))[0]
        sup.append((int(rows.min()), int(rows.max())))

    slice_offs, blocks = [], []
    for c in range(NCORES):
        tiles = CORE_TILES[c]
        los = [sup[j][0] for j in tiles]
        his = [sup[j][1] for j in tiles]
        offs = []
        for i in range(NSL):
            o = L - 128
            if i < NTPC:
                o = min(o, los[i])
            if i > 0:
                o = min(o, offs[i - 1] + 128)
            offs.append(max(0, o))
        for i in range(NTPC):
            assert offs[i] <= los[i] and offs[i + 2] + 128 > his[i], \
                (c, i, offs, los, his)

        blks = []
        for t, j in enumerate(tiles):
            covered = np.zeros(L + 128, bool)
            for b in range(NDP[t]):
                o = offs[SMAP[t][b]]
                new = ~covered[o:o + 128]
                blks.append((j, o, new.copy()))
                covered[o:o + 128] = True
            assert covered[sup[j][0]:sup[j][1] + 1].all()
        slice_offs.append(offs)
        blocks.append(blks)
    return slice_offs, blocks


def _shard_inputs(x, A):
    """Per-core in_maps plus host-side residual rows (lin 4096..4099)."""
    X = np.ascontiguousarray(
        np.asarray(x, np.float32).transpose(3, 1, 0, 2).reshape(L, R))
    sx = SX_TARGET / max(float(np.abs(X).max()), 1e-30)
    Xq = np.clip(X * sx, -15.5, 15.5).astype(F8)

    slice_offs, blocks = _plan_slices(A)
    in_maps = []
    for c in range(NCORES):
        # (chunk, slice)-interleaved column layout matching _build_program
        xin = np.zeros((128, NSL * R), F8)
        for s, o in enumerate(slice_offs[c]):
            for ch in range(NCH):
                d0 = (ch * NSL + s) * CHUNK
                xin[:, d0:d0 + CHUNK] = Xq[o:o + 128,
                                           ch * CHUNK:(ch + 1) * CHUNK]
        wts = np.zeros((128, NBLK * 128), np.float32)
        for bi, (j, o, new) in enumerate(blocks[c]):
            wblk = A[o:o + 128, j * 128:(j + 1) * 128] * new[:, None]
            wts[:, bi * 128:(bi + 1) * 128] = wblk
        wts = (wts / sx).astype(np.float16)
        in_maps.append({"xin": xin, "wts": wts})

    # host residual: out lins [4096, 4100) (f-bin 1024), exact in fp32
    nzc = A[:L, RES_LO:L] != 0
    ri = int(np.nonzero(nzc.any(axis=1))[0].min())
    residual = A[ri:L, RES_LO:L].T @ X[ri:L]             # [4, R] fp32
    return in_maps, residual


def _gather_output(results, bias_img, residual):
    out_lin = np.zeros((L, R), np.float32)
    for c in range(NCORES):
        o8 = np.asarray(results[c]["out8"]).astype(np.float32) / SO
        for t, j in enumerate(CORE_TILES[c]):
            out_lin[j * 128:(j + 1) * 128] = o8[t * 128:(t + 1) * 128]
    out_lin[RES_LO:L] = residual
    out = out_lin.reshape(F, C, B, T).transpose(2, 1, 3, 0)
    out = np.ascontiguousarray(out) + bias_img[None, :, None, :]
    return out.astype(np.float32)


def _run_on_device(in_maps, loop_iters=1):
    from concourse.bass_utils import run_bass_kernel_spmd
    nc = _build_program(loop_iters)
    res = run_bass_kernel_spmd(nc, in_maps, list(range(NCORES)))
    return res.results


def kernel(x, pre_weight, pre_bias, post_weight, post_bias, mask, ola_window,
           f_idxes):
    x = np.asarray(x, np.float32)
    pre_weight = np.asarray(pre_weight, np.float32)
    pre_bias = np.asarray(pre_bias, np.float32)
    post_weight = np.asarray(post_weight, np.float32)
    post_bias = np.asarray(post_bias, np.float32)
    mask = np.asarray(mask, np.float32)
    ola_window = np.asarray(ola_window, np.float32)
    f_idxes = np.asarray(f_idxes)

    A, bias_img = _build_A(pre_weight, pre_bias, post_weight, post_bias,
                           mask, ola_window, f_idxes)
    in_maps, residual = _shard_inputs(x, A)
    results = _run_on_device(in_maps)
    return _gather_output(results, bias_img, residual)
